# revision 1
# baseline (speedup 1.0000x reference)
"""Pairwise Euclidean distance matrix on 8 TRN2 NeuronCores (Bass/Tile).

out[i, j] = ||x[j] - x[i]||_2 for x [4096, 512] fp32.

Distance symmetry: out = out.T, so only ~half the blocks are computed.
Half-ring decomposition: core c owns query (column) block c and computes
it against key (row) blocks {c, c+1, .., c+4 mod 8} — 5 of 8 blocks,
perfectly balanced and SPMD-uniform. Blocks at ring distance 1..3 are
mirrored into their transposed position on the host during unsharding;
distance 0/4 positions are covered directly.

d2 = sq[i] + sq[j] - 2*x[i].x[j] via PE matmuls. The Gram part runs as a
split-bf16 product (x = hi + lo in bf16; hi.hi + hi.lo + lo.hi
accumulated into the same fp32 PSUM tile) — fp32-class accuracy at bf16
speed. Queries are pre-scaled by -2 on host (exact in bf16), so
PSUM = d2 - sq_m - sq_n; epilogue: DVE adds sq_m (replicated over
partitions), ACT computes Sqrt(x + sq_n) with sq_n as per-partition
bias. The diagonal (d2 == 0 exactly) is zeroed on host.
"""

import numpy as np
import ml_dtypes

import concourse.bass as bass
import concourse.bacc as bacc
import concourse.tile as tile
from concourse.bass_utils import run_bass_kernel_spmd

mybir = bass.mybir

N = 4096          # number of points
D = 512           # feature dim
NCORES = 8
QB = N // NCORES  # 512 queries per core
KT = D // 128     # 4 contraction tiles
RB = 5            # row blocks per core (half-ring)
NT = RB * QB // 128   # 20 key tiles of 128 per core
KEYS = RB * QB        # 2560 keys per core
CG = [512, 1024, 1024]  # key column grouping for DMA staging

_BF16 = mybir.dt.bfloat16
_F32 = mybir.dt.float32

_nc_cache = {}


def _build():
    if "nc" in _nc_cache:
        return _nc_cache["nc"]
    nc = bacc.Bacc("TRN2", target_bir_lowering=False, debug=False)

    # keys: hi block then lo block along the column axis
    xp = nc.dram_tensor("xp", [D, 2 * KEYS], _BF16, kind="ExternalInput")
    # queries: hi and lo halves packed side by side, pre-scaled by -2
    q = nc.dram_tensor("q", [D, 2 * QB], _BF16, kind="ExternalInput")
    # squared norms: cols 0:NT per-key-tile table, NT:NT+QB query row
    sq = nc.dram_tensor("sq", [128, NT + QB], _F32, kind="ExternalInput")
    out = nc.dram_tensor("out", [KEYS, QB], _F32, kind="ExternalOutput")

    xp4 = xp.ap().rearrange("(k p) (t n) -> p k t n", p=128, t=2)  # [128,4,2,KEYS]

    with tile.TileContext(nc) as tc:
        with (
            tc.tile_pool(name="xd", bufs=1) as xd,
            tc.tile_pool(name="op", bufs=4) as op,
            tc.tile_pool(name="ps", bufs=8, space="PSUM") as pp,
        ):
            # DMA triggers cost ~640ns each and serialize per engine, so
            # spread them: queries on sync, sq tables on scalar, keys on
            # gpsimd.
            t_q = []
            for k in range(KT):
                t = xd.tile([128, 2 * QB], _BF16, tag=f"q{k}", name=f"q{k}")
                nc.sync.dma_start(t[:], q.ap()[k * 128 : (k + 1) * 128, :])
                t_q.append(t)
            t_qh = [t[:, 0:QB] for t in t_q]
            t_ql = [t[:, QB : 2 * QB] for t in t_q]

            t_sq = xd.tile([128, NT + QB], _F32, tag="sq", name="sq")
            nc.scalar.dma_start(t_sq[:], sq.ap())
            t_sqn = t_sq[:, 0:NT]
            t_sqm = t_sq[:, NT : NT + QB]

            # The PE sits idle while the first DMAs land, leaving the HAM
            # clock gate cold (1.2 GHz) for the first ~3.4us of real
            # matmuls. Warm it with dummy matmuls on a memset tile; the
            # PSUM slot comes from the shared pool and is recycled.
            warm = xd.tile([128, QB], _BF16, tag="warm", name="warm")
            nc.vector.memset(warm[:], 0.0)
            wps = pp.tile([128, QB], _F32, tag="ps", name="wps")
            for _ in range(10):
                nc.tensor.matmul(
                    wps[:], warm[:, 0:128], warm[:], start=True, stop=True
                )

            # key tiles: one full-width [128, KEYS] tile per (hi/lo, k).
            # Full rows give 5KB descriptor runs (full DMA bandwidth); hi
            # tiles load before lo tiles, matching consumption order. The
            # k0-hi tile is split so the first matmul group only waits on
            # its own 256KB half.
            t_hi, t_lo = [None], []
            hi0a = xd.tile([128, 1024], _BF16, tag="hi0a", name="hi0a")
            nc.gpsimd.dma_start(hi0a[:], xp4[:, 0, 0, 0:1024])
            for k in range(1, KT):
                t = xd.tile(
                    [128, KEYS], _BF16, tag=f"x0_{k}", name=f"x0_{k}"
                )
                nc.gpsimd.dma_start(t[:], xp4[:, k, 0, :])
                t_hi.append(t)
            hi0b = xd.tile([128, KEYS - 1024], _BF16, tag="hi0b", name="hi0b")
            nc.gpsimd.dma_start(hi0b[:], xp4[:, 0, 0, 1024:KEYS])
            for k in range(KT):
                t = xd.tile(
                    [128, KEYS], _BF16, tag=f"x1_{k}", name=f"x1_{k}"
                )
                nc.gpsimd.dma_start(t[:], xp4[:, k, 1, :])
                t_lo.append(t)

            def hi_slice(k, j):
                if k == 0:
                    if j < 8:
                        return hi0a[:, j * 128 : (j + 1) * 128]
                    return hi0b[:, j * 128 - 1024 : (j + 1) * 128 - 1024]
                return t_hi[k][:, j * 128 : (j + 1) * 128]

            sqrt = mybir.ActivationFunctionType.Sqrt
            pair_tile = {}

            def epilogue(j, p):
                # paired output: two row-tiles share one [128, 1024] tile
                # and one DMA (3D DRAM access pattern). Pair triggers
                # alternate sync/scalar so the final two fire in parallel
                # instead of serializing ~650ns apiece on one engine.
                jp, half = j // 2, j % 2
                if half == 0:
                    pair_tile[jp] = op.tile(
                        [128, 2 * QB], _F32, tag="o", name=f"o{jp}"
                    )
                o = pair_tile[jp]
                sl = slice(half * QB, (half + 1) * QB)
                nc.vector.tensor_add(o[:, sl], p[:], t_sqm)
                nc.scalar.activation(
                    o[:, sl], o[:, sl], sqrt,
                    bias=t_sqn[:, j : j + 1], scale=1.0,
                )
                if half == 1:
                    dst = (
                        out.ap()[(j - 1) * 128 : (j + 1) * 128, :]
                        .rearrange("(c p) n -> p c n", p=128)
                    )
                    src = o[:].rearrange("p (c n) -> p c n", c=2)
                    eng = nc.sync if jp % 2 == 0 else nc.scalar
                    eng.dma_start(dst, src)

            # Groups of 8 key tiles (= PSUM banks). Within a group the hi
            # phases run k-outer so the PE starts on the first hi k-tile
            # while later ones stream in; the lo phase runs j-inner so
            # early PSUM tiles complete (and free their bank) before the
            # group sweep ends.
            for g0 in range(0, NT, 8):
                js = range(g0, min(g0 + 8, NT))
                psums = {
                    j: pp.tile([128, QB], _F32, tag="ps", name=f"ps{j}")
                    for j in js
                }
                for k in range(KT):
                    for j in js:
                        w = hi_slice(k, j)
                        nc.tensor.matmul(
                            psums[j][:], w, t_qh[k][:], start=(k == 0), stop=False
                        )
                        nc.tensor.matmul(
                            psums[j][:], w, t_ql[k][:], start=False, stop=False
                        )
                for j in js:
                    for k in range(KT):
                        nc.tensor.matmul(
                            psums[j][:],
                            t_lo[k][:, j * 128 : (j + 1) * 128],
                            t_qh[k][:],
                            start=False,
                            stop=(k == KT - 1),
                        )
                    epilogue(j, psums[j])

    nc.compile()
    _nc_cache["nc"] = nc
    return nc


def _ring(c):
    return [(c + t) % NCORES for t in range(RB)]


def _prep_inputs(x: np.ndarray):
    x = np.ascontiguousarray(x, dtype=np.float32)
    xh16 = x.astype(ml_dtypes.bfloat16)
    xh32 = xh16.astype(np.float32)
    xl16 = (x - xh32).astype(ml_dtypes.bfloat16)
    xl32 = xl16.astype(np.float32)

    xe = xh32.astype(np.float64) + xl32.astype(np.float64)
    sqv = np.einsum("nd,nd->n", xe, xe)

    xhT = np.ascontiguousarray(xh16.T)  # [D, N]
    xlT = np.ascontiguousarray(xl16.T)

    in_maps = []
    for c in range(NCORES):
        r0 = c * QB
        rows = _ring(c)
        keycols = np.concatenate([np.arange(r * QB, (r + 1) * QB) for r in rows])
        sq_keys = sqv[keycols].astype(np.float32)
        sq_pack = np.concatenate(
            [
                sq_keys.reshape(NT, 128).T,  # [128, NT]
                np.broadcast_to(sqv[r0 : r0 + QB].astype(np.float32), (128, QB)),
            ],
            axis=1,
        )
        in_maps.append(
            {
                "xp": np.ascontiguousarray(
                    np.concatenate([xhT[:, keycols], xlT[:, keycols]], axis=1)
                ),
                "q": np.ascontiguousarray(
                    np.concatenate(
                        [
                            (-2.0 * xh32[r0 : r0 + QB]).astype(ml_dtypes.bfloat16).T,
                            (-2.0 * xl32[r0 : r0 + QB]).astype(ml_dtypes.bfloat16).T,
                        ],
                        axis=1,
                    )
                ),
                "sq": np.ascontiguousarray(sq_pack),
            }
        )
    return in_maps


def run(x: np.ndarray, trace: bool = False, tmpdir: str | None = None):
    nc = _build()
    in_maps = _prep_inputs(x)
    res = run_bass_kernel_spmd(
        nc, in_maps, list(range(NCORES)), trace=trace, tmpdir=tmpdir
    )
    full = np.empty((N, N), dtype=np.float32)
    for c in range(NCORES):
        blk = res.results[c]["out"]  # [KEYS, QB]
        for t, r in enumerate(_ring(c)):
            b = blk[t * QB : (t + 1) * QB, :]  # rows r*QB.., cols c*QB..
            full[r * QB : (r + 1) * QB, c * QB : (c + 1) * QB] = b
            if t in (1, 2, 3):  # ring distance 1..3: mirror transpose
                full[c * QB : (c + 1) * QB, r * QB : (r + 1) * QB] = b.T
    np.fill_diagonal(full, 0.0)
    return full, res


def kernel(x: np.ndarray) -> np.ndarray:
    out, _ = run(x, trace=False)
    return out



# revision 3
# speedup vs baseline: 1.8013x; 1.8013x over previous
"""Pairwise Euclidean distance matrix on 8 TRN2 NeuronCores (Bass/Tile).

out[i, j] = ||x[j] - x[i]||_2 for x [4096, 512] fp32.

Distance symmetry: out = out.T, so only ~half the blocks are computed.
Half-ring decomposition: core c owns query (column) block c and computes
it against key (row) blocks {c, c+1, .., c+4 mod 8} — 5 of 8 blocks,
perfectly balanced and SPMD-uniform. Blocks at ring distance 1..3 are
mirrored into their transposed position on the host during unsharding;
distance 0/4 positions are covered directly.

d2 = sq[i] + sq[j] - 2*x[i].x[j]. The Gram part runs as fp8 e4m3
matmuls in DoubleRow perf mode (2 fp8 weights/cell, 2 MACs/cycle):
keys and -2*queries are quantized to e4m3 on host; the quantization
error lands ~6.5e-3 on the harness metric (gate 2e-2). PSUM holds
-2*dot in f32. Epilogue per 4-bank group: one DVE scalar_tensor_tensor
computes bf16(psum + sq_n(per-partition) + sq_m(broadcast tile)), one
big ACT Sqrt converts d2->d in place, and one DMA stores the [512,512]
f32->bf16 block. Host upcasts, mirrors, and zeroes the diagonal
(diagonal d2 can go slightly negative under fp8 -> NaN, overwritten).
"""

import numpy as np
import ml_dtypes

import concourse.bass as bass
import concourse.bacc as bacc
import concourse.tile as tile
from concourse.bass_utils import run_bass_kernel_spmd

mybir = bass.mybir

N = 4096          # number of points
D = 512           # feature dim
NCORES = 8
QB = N // NCORES  # 512 queries per core
RB = 5            # row blocks per core (half-ring)
NT = RB * QB // 128   # 20 key tiles of 128 per core
KEYS = RB * QB        # 2560 keys per core
GT = 4                # key tiles per PSUM group (4 banks)
NG = NT // GT         # 5 groups

_FP8 = mybir.dt.float8e4
_BF16 = mybir.dt.bfloat16
_F32 = mybir.dt.float32

_nc_cache = {}


def _build():
    if "nc" in _nc_cache:
        return _nc_cache["nc"]
    nc = bacc.Bacc("TRN2", target_bir_lowering=False, debug=False)

    # keys: [D, KEYS] e4m3, feature-major
    xp = nc.dram_tensor("xp", [D, KEYS], _FP8, kind="ExternalInput")
    # queries: [D, QB] e4m3, pre-scaled by -2
    q = nc.dram_tensor("q", [D, QB], _FP8, kind="ExternalInput")
    # squared norms (f32): cols 0:NT per-key-tile table, NT:NT+QB query row
    sq = nc.dram_tensor("sq", [128, NT + QB], _F32, kind="ExternalInput")
    out = nc.dram_tensor("out", [KEYS, QB], _BF16, kind="ExternalOutput")

    # [128, k-subtile, n] views: partition = feature%128, dim1 = feature/128
    xp4 = xp.ap().rearrange("(k p) n -> p k n", p=128)   # [128, 4, KEYS]
    q4s = q.ap().rearrange("(k p) n -> p k n", p=128)    # [128, 4, QB]

    sqrt = mybir.ActivationFunctionType.Sqrt
    add = mybir.AluOpType.add

    with tile.TileContext(nc) as tc:
        with (
            tc.tile_pool(name="xd", bufs=1) as xd,
            tc.tile_pool(name="op", bufs=3) as op,
            tc.tile_pool(name="ps", bufs=2, space="PSUM") as pp,
        ):
            # ---- input DMAs (triggers cost ~640ns and serialize per
            # engine; first-needed data goes on scalar, rest on sync) ----
            # keys, as two k-pair tiles for DoubleRow [128,2,*] slices;
            # kp0 split so group 0 only waits on its own 0.33MB.
            kp0a = xd.tile([128, 2, 1024], _FP8, tag="kp0a", name="kp0a")
            nc.scalar.dma_start(kp0a[:], xp4[:, 0:2, 0:1024])
            kp1a = xd.tile([128, 2, 1024], _FP8, tag="kp1a", name="kp1a")
            nc.scalar.dma_start(kp1a[:], xp4[:, 2:4, 0:1024])

            # ACT sqrt table preload (~2.7us) while PE warms / DMAs land
            dumm = xd.tile([128, 1], _F32, tag="dumm", name="dumm")
            nc.vector.memset(dumm[:], 1.0)
            nc.scalar.activation(dumm[:], dumm[:], sqrt, bias=0.0, scale=1.0)

            t_q = xd.tile([128, 4, QB], _FP8, tag="q", name="q")
            nc.sync.dma_start(t_q[:], q4s)

            t_sq = xd.tile([128, NT + QB], _F32, tag="sq", name="sq")
            nc.sync.dma_start(t_sq[:], sq.ap())
            t_sqn = t_sq[:, 0:NT]            # per-key-tile norms
            t_sqm = t_sq[:, NT : NT + QB]    # query norms, bcast over parts

            kp0b = xd.tile([128, 2, KEYS - 1024], _FP8, tag="kp0b", name="kp0b")
            nc.sync.dma_start(kp0b[:], xp4[:, 0:2, 1024:KEYS])
            kp1b = xd.tile([128, 2, KEYS - 1024], _FP8, tag="kp1b", name="kp1b")
            nc.sync.dma_start(kp1b[:], xp4[:, 2:4, 1024:KEYS])

            def key_slice(kp, j):
                lo, hi = j * 128, (j + 1) * 128
                if lo < 1024:
                    t = kp0a if kp == 0 else kp1a
                    return t[:, :, lo:hi]
                t = kp0b if kp == 0 else kp1b
                return t[:, :, lo - 1024 : hi - 1024]

            # PE warmup: HAM clock gate is cold (1.2 GHz) for ~3.4us of
            # sustained activity; burn it on dummies while DMAs land.
            warm = xd.tile([128, QB], _BF16, tag="warm", name="warm")
            nc.vector.memset(warm[:], 0.0)
            # shares the ps pool round-robin (PSUM is exactly 2 bufs wide)
            wps = pp.tile([128, GT * QB], _F32, tag="ps", name="wps")
            for _ in range(4):
                nc.tensor.matmul(
                    wps[:, 0:QB], warm[:, 0:128], warm[:], start=True, stop=True
                )

            dr = mybir.MatmulPerfMode.DoubleRow
            for g in range(NG):
                js = range(g * GT, (g + 1) * GT)
                psg = pp.tile([128, GT * QB], _F32, tag="ps", name=f"ps{g}")
                for kp in (0, 1):
                    for i, j in enumerate(js):
                        nc.tensor.matmul(
                            psg[:, i * QB : (i + 1) * QB],
                            key_slice(kp, j),
                            t_q[:, 2 * kp : 2 * kp + 2, :],
                            start=(kp == 0),
                            stop=(kp == 1),
                            perf_mode=dr,
                        )
                o = op.tile([128, GT * QB], _BF16, tag="o", name=f"o{g}")
                # d2 = psum + sq_n (per-partition scalar) + sq_m (bcast)
                for i, j in enumerate(js):
                    sl = slice(i * QB, (i + 1) * QB)
                    nc.vector.scalar_tensor_tensor(
                        o[:, sl], psg[:, sl], t_sqn[:, j : j + 1], t_sqm,
                        add, add,
                    )
                nc.scalar.activation(o[:], o[:], sqrt, bias=0.0, scale=1.0)
                dst = (
                    out.ap()[g * GT * 128 : (g + 1) * GT * 128, :]
                    .rearrange("(c p) n -> p c n", p=128)
                )
                src = o[:].rearrange("p (c n) -> p c n", c=GT)
                nc.sync.dma_start(dst, src)

    nc.compile()
    _nc_cache["nc"] = nc
    return nc


def _ring(c):
    return [(c + t) % NCORES for t in range(RB)]


def _prep_inputs(x: np.ndarray):
    x = np.ascontiguousarray(x, dtype=np.float32)
    x8 = x.astype(ml_dtypes.float8_e4m3)
    q8 = (-2.0 * x).astype(ml_dtypes.float8_e4m3)

    sqv = np.einsum("nd,nd->n", x.astype(np.float64), x.astype(np.float64))

    x8T = np.ascontiguousarray(x8.T)  # [D, N]
    q8T = np.ascontiguousarray(q8.T)

    in_maps = []
    for c in range(NCORES):
        r0 = c * QB
        rows = _ring(c)
        keycols = np.concatenate([np.arange(r * QB, (r + 1) * QB) for r in rows])
        sq_keys = sqv[keycols].astype(np.float32)
        sq_pack = np.concatenate(
            [
                sq_keys.reshape(NT, 128).T,  # [128, NT]
                np.broadcast_to(sqv[r0 : r0 + QB].astype(np.float32), (128, QB)),
            ],
            axis=1,
        )
        in_maps.append(
            {
                "xp": np.ascontiguousarray(x8T[:, keycols]),
                "q": np.ascontiguousarray(q8T[:, r0 : r0 + QB]),
                "sq": np.ascontiguousarray(sq_pack),
            }
        )
    return in_maps


def run(x: np.ndarray, trace: bool = False, tmpdir: str | None = None):
    nc = _build()
    in_maps = _prep_inputs(x)
    res = run_bass_kernel_spmd(
        nc, in_maps, list(range(NCORES)), trace=trace, tmpdir=tmpdir
    )
    full = np.empty((N, N), dtype=np.float32)
    for c in range(NCORES):
        blk = res.results[c]["out"].astype(np.float32)  # [KEYS, QB]
        for t, r in enumerate(_ring(c)):
            b = blk[t * QB : (t + 1) * QB, :]  # rows r*QB.., cols c*QB..
            full[r * QB : (r + 1) * QB, c * QB : (c + 1) * QB] = b
            if t in (1, 2, 3):  # ring distance 1..3: mirror transpose
                full[c * QB : (c + 1) * QB, r * QB : (r + 1) * QB] = b.T
    np.fill_diagonal(full, 0.0)
    return full, res


def kernel(x: np.ndarray) -> np.ndarray:
    out, _ = run(x, trace=False)
    return out


# revision 4
# speedup vs baseline: 2.0477x; 1.1368x over previous
"""Pairwise Euclidean distance matrix on 8 TRN2 NeuronCores (Bass/Tile).

out[i, j] = ||x[j] - x[i]||_2 for x [4096, 512] fp32.

Distance symmetry: out = out.T, so only ~half the blocks are computed.
Half-ring decomposition: core c owns query block c and computes it
against key blocks {c, c+1, .., c+4 mod 8} — 5 of 8 blocks, perfectly
balanced and SPMD-uniform. Ring distance 1..3 blocks are mirrored into
their transposed position on the host; distance 0/4 covered directly.

Layout: queries on PSUM partitions, keys on the free axis. The Gram
part runs as fp8 e4m3 DoubleRow matmuls (2 fp8 weights/cell, 2 MACs/
cycle): query subblocks [128c,2,128q] stationary, key chunks
[128c,2,512k] moving; -2 is pre-folded into the quantized queries. A
tiny [2,128]x[2,512] bf16 augmentation matmul per PSUM tile adds
sq_m (per-query) + sq_n (per-key) into the same accumulation, so PSUM
holds d^2 directly. Epilogue per 4-bank group is then a single big ACT
Sqrt (PSUM -> SBUF bf16) and one output DMA in SBUF-native layout
(host unscrambles). No DVE work at all; quantization error lands
~7e-3 on the harness metric (gate 2e-2). Diagonal d^2 can go slightly
negative under fp8 -> NaN after sqrt; host overwrites the diagonal.
"""

import numpy as np
import ml_dtypes

import concourse.bass as bass
import concourse.bacc as bacc
import concourse.tile as tile
from concourse.bass_utils import run_bass_kernel_spmd

mybir = bass.mybir

N = 4096          # number of points
D = 512           # feature dim
NCORES = 8
QB = N // NCORES  # 512 queries per core
RB = 5            # row blocks per core (half-ring)
KEYS = RB * QB    # 2560 keys per core
NC = 5            # key chunks of 512 per core
NS = 4            # query subblocks of 128

_FP8 = mybir.dt.float8e4
_BF16 = mybir.dt.bfloat16
_F32 = mybir.dt.float32

_nc_cache = {}


def _build():
    if "nc" in _nc_cache:
        return _nc_cache["nc"]
    nc = bacc.Bacc("TRN2", target_bir_lowering=False, debug=False)

    # keys: [128, chunk, k-subtile, 512] e4m3 packed host-side so each
    # chunk DMA reads 2048B contiguous per partition
    xp = nc.dram_tensor("xp", [128, NC * 4 * 512], _FP8, kind="ExternalInput")
    # queries: [128, k-subtile, 512] e4m3, pre-scaled by -2
    q = nc.dram_tensor("q", [128, 4 * QB], _FP8, kind="ExternalInput")
    # augmentation rows (bf16): cols 0:QB = (sq_m; ones),
    # cols QB:QB+KEYS = (ones; sq_n)
    aug = nc.dram_tensor("aug", [2, QB + KEYS], _BF16, kind="ExternalInput")
    # output in SBUF-native layout: col block t=c*4+s holds
    # d[query s*128+p, key chunk c]
    out = nc.dram_tensor("out", [128, NC * NS * 512], _BF16, kind="ExternalOutput")

    sqrt = mybir.ActivationFunctionType.Sqrt
    dr = mybir.MatmulPerfMode.DoubleRow

    with tile.TileContext(nc) as tc:
        with (
            tc.tile_pool(name="xd", bufs=1) as xd,
            tc.tile_pool(name="op", bufs=3) as op,
            tc.tile_pool(name="ps", bufs=2, space="PSUM") as pp,
        ):
            # ACT sqrt table preload (~2.7us) rides the startup phase
            dumm = xd.tile([128, 1], _F32, tag="dumm", name="dumm")
            nc.vector.memset(dumm[:], 1.0)
            nc.scalar.activation(dumm[:], dumm[:], sqrt, bias=0.0, scale=1.0)

            # aug rows early on gpsimd (its only load)
            t_aug = xd.tile([2, QB + KEYS], _BF16, tag="aug", name="aug")
            nc.gpsimd.dma_start(t_aug[:], aug.ap())

            # key chunks + queries on sync, first-needed first
            t_k = []
            for c in range(NC):
                t = xd.tile([128, 2, 2, 512], _FP8, tag=f"k{c}", name=f"k{c}")
                src = xp.ap()[:, c * 2048 : (c + 1) * 2048].rearrange(
                    "p (kp k n) -> p kp k n", kp=2, k=2
                )
                nc.sync.dma_start(t[:], src)
                t_k.append(t)
                if c == 0:
                    t_q = xd.tile([128, 2, 2, QB], _FP8, tag="q", name="q")
                    nc.sync.dma_start(
                        t_q[:],
                        q.ap().rearrange("p (kp k n) -> p kp k n", kp=2, k=2),
                    )

            # PE warmup: HAM clock gate is cold (1.2 GHz) until ~3.4us of
            # sustained activity; burn the wait for the first key chunk.
            warm = xd.tile([128, QB], _BF16, tag="warm", name="warm")
            nc.vector.memset(warm[:], 0.0)
            wps = pp.tile([128, NS * 512], _F32, tag="ps", name="wps")
            for _ in range(6):
                nc.tensor.matmul(
                    wps[:, 0:QB], warm[:, 0:128], warm[:], start=True, stop=True
                )

            for c in range(NC):
                psg = pp.tile([128, NS * 512], _F32, tag="ps", name=f"ps{c}")
                for kp in (0, 1):
                    for s in range(NS):
                        nc.tensor.matmul(
                            psg[:, s * 512 : (s + 1) * 512],
                            t_q[:, kp, :, s * 128 : (s + 1) * 128],
                            t_k[c][:, kp, :, :],
                            start=(kp == 0),
                            stop=False,
                            perf_mode=dr,
                        )
                for s in range(NS):
                    nc.tensor.matmul(
                        psg[:, s * 512 : (s + 1) * 512],
                        t_aug[:, s * 128 : (s + 1) * 128],
                        t_aug[:, QB + c * 512 : QB + (c + 1) * 512],
                        start=False,
                        stop=True,
                    )
                o = op.tile([128, NS * 512], _BF16, tag="o", name=f"o{c}")
                nc.scalar.activation(o[:], psg[:], sqrt, bias=0.0, scale=1.0)
                nc.gpsimd.dma_start(
                    out.ap()[:, c * NS * 512 : (c + 1) * NS * 512], o[:]
                )

    nc.compile()
    _nc_cache["nc"] = nc
    return nc


def _ring(c):
    return [(c + t) % NCORES for t in range(RB)]


def _prep_inputs(x: np.ndarray):
    x = np.ascontiguousarray(x, dtype=np.float32)
    x8 = x.astype(ml_dtypes.float8_e4m3)       # keys [N, D]
    q8 = (-2.0 * x).astype(ml_dtypes.float8_e4m3)
    sqv = np.einsum("nd,nd->n", x.astype(np.float64), x.astype(np.float64))
    sqb = sqv.astype(ml_dtypes.bfloat16)
    ones = np.ones(N, dtype=ml_dtypes.bfloat16)

    in_maps = []
    for c in range(NCORES):
        r0 = c * QB
        keycols = np.concatenate(
            [np.arange(r * QB, (r + 1) * QB) for r in _ring(c)]
        )
        # keys: [p, chunk, ksub, n] with feature 128*ksub+p of key keycols[.]
        kc = x8[keycols, :].reshape(NC, 512, 4, 128)  # [c, n, k, p]
        xp_pack = kc.transpose(3, 0, 2, 1).reshape(128, NC * 4 * 512)
        # queries: [p, ksub, j]
        qc = q8[r0 : r0 + QB, :].reshape(QB, 4, 128)
        q_pack = qc.transpose(2, 1, 0).reshape(128, 4 * QB)
        aug_pack = np.empty((2, QB + KEYS), dtype=ml_dtypes.bfloat16)
        aug_pack[0, 0:QB] = sqb[r0 : r0 + QB]
        aug_pack[1, 0:QB] = ones[0:QB]
        aug_pack[0, QB:] = ones[0:KEYS]
        aug_pack[1, QB:] = sqb[keycols]
        in_maps.append(
            {
                "xp": np.ascontiguousarray(xp_pack),
                "q": np.ascontiguousarray(q_pack),
                "aug": np.ascontiguousarray(aug_pack),
            }
        )
    return in_maps


def run(x: np.ndarray, trace: bool = False, tmpdir: str | None = None):
    nc = _build()
    in_maps = _prep_inputs(x)
    res = run_bass_kernel_spmd(
        nc, in_maps, list(range(NCORES)), trace=trace, tmpdir=tmpdir
    )
    full = np.empty((N, N), dtype=np.float32)
    for c in range(NCORES):
        o = res.results[c]["out"].astype(np.float32)
        # [p, c, s, n] -> blk[q = s*128+p, key = c*512+n]
        blk = o.reshape(128, NC, NS, 512).transpose(2, 0, 1, 3).reshape(QB, KEYS)
        for t, r in enumerate(_ring(c)):
            b = blk[:, t * QB : (t + 1) * QB]  # [queries blk c, keys blk r]
            full[r * QB : (r + 1) * QB, c * QB : (c + 1) * QB] = b.T
            if t in (1, 2, 3):  # ring distance 1..3: mirror
                full[c * QB : (c + 1) * QB, r * QB : (r + 1) * QB] = b
    np.fill_diagonal(full, 0.0)
    return full, res


def kernel(x: np.ndarray) -> np.ndarray:
    out, _ = run(x, trace=False)
    return out
